# revision 29
# baseline (speedup 1.0000x reference)
"""Trainium2 Bass kernel for nn_ChannelWiseConv (depthwise conv stack + KAN head).

Strategy (per core, pure data parallelism over batch):
  - 256 images/core in bf16. Each stride-2 depthwise 3x3 conv is computed as 3
    PE matmuls accumulating in PSUM: contraction over input rows with
    per-channel banded weight matrices (one per kernel-column tap); the column
    subsampling is expressed in the moving-operand access pattern. Channels are
    blocked (2/4/8/13 per matmul as planes shrink) to keep K near 128.
  - The six conv stages are software-pipelined across image groups (stage s of
    group g runs in wave g+s) so the in-order PE queue never waits on a
    same-group PSUM drain; bias+relu drains are spread across ACT/DVE/Pool.
  - Constants ship as four packed DMAs (HWDGE overhead is ~625ns per DMA);
    groups 0-1 are chunked per channel-pair so the first matmuls start
    ~1.5us in instead of waiting for full 1.6MB group transfers.
  - KAN head: the cubic B-spline basis is evaluated in closed form via the
    truncated-power identity 6*B_n(u) = sum_k (-1)^k C(4,k) relu(u-n-k)^3,
    fused to three wide ops: t[:,k*13+i] = u_i - k via a single
    stride-0-broadcast tensor_tensor add against a host-packed -k table,
    sq = t*t (sign-free), cube = (t max 0)*sq as one scalar_tensor_tensor,
    then two shifted linear combinations. This replaces the serial
    Cox-de-Boor recursion; /6 and spline scales fold into the host-packed
    coefficient matrices. One PE matmul per KAN layer
    over [silu(x); basis; 1] stacks. Four 64-image chains are pipelined INTO
    the conv waves (chain c starts once its t_u chunk is drained). silu is
    computed as x*1/(1+exp(-x)) (ACT exp + DVE fast reciprocal) because Silu
    shares no ACT function table with Exp: the whole program then needs only
    one table swap, for the 4 deferred Ln ops of log_softmax at the end.
    Pool (GPSIMD) never touches PSUM and never runs scalar_tensor_tensor --
    the BIR verifier rejects both.
  - log_softmax on-chip; all chains write one [64, 4*10] tile -> single
    output DMA; output [256, 10] per core, concatenated on host.
"""

import ml_dtypes
import numpy as np

BF16 = ml_dtypes.bfloat16

IN_CH, HIDDEN, NCLS = 13, 20, 10
B_FULL, NCORE = 2048, 8
B = B_FULL // NCORE          # images per core
NG = 16                      # image groups per core
GI = B // NG                 # images per group (16)
H_GRID = 0.4                 # KAN knot spacing; u = (x + 2.2) / 0.4
NCH = 5                      # KAN chains per core (last two smaller: the
CL = 64                      # tail chain spans 1 group and starts a wave early)
CH_START = (0, 64, 128, 192, 240)
CH_N = (64, 64, 64, 48, 16)
CH_W0 = (9, 13, 17, 20, 21)

CG1 = [(0, 2), (2, 2), (4, 2), (6, 2), (8, 2), (10, 2), (12, 1)]
CG2 = [(0, 4), (4, 4), (8, 4), (12, 1)]
CG3 = [(0, 8), (8, 5)]

# packed-constant column layouts: name -> (col0, ncols, rows)
CEA = {"bv1": (0, 7, 64), "bv2": (7, 4, 64), "bv3": (11, 2, 64),
       "bv4": (13, 1, 52), "bv5": (14, 1, 26), "ubias": (15, 1, 13),
       "misc": (16, 3, 128)}
CEB = {"bands1": (0, 1344, 128)}
CEC = {"bands2": (0, 768, 128), "bands3": (768, 384, 128),
       "bands4": (1152, 156, 104), "bands5": (1308, 78, 52),
       "bands6": (1386, 26, 26)}
CED = {"negk": (0, 12, 128), "iden": (12, 128, 128), "c1a": (140, 20, 13),
       "c1b": (160, 20, 105), "c2s": (180, 10, 20), "c2b1": (190, 10, 108),
       "c2b2": (200, 10, 53)}
CEA_N, CEB_N, CEC_N, CED_N = 19, 1344, 1412, 210

_BUILT = None  # cached (nc, input_names)


# ----------------------------------------------------------------------------
# host-side constant packing
# ----------------------------------------------------------------------------

def _pack_bands(w, S, cgs, slotM, rows):
    So = S // 2
    out = np.zeros((rows, len(cgs) * 3 * slotM), np.float32)
    for gi, (c0, nch) in enumerate(cgs):
        for b in range(3):
            col0 = (gi * 3 + b) * slotM
            for cl in range(nch):
                for i in range(So):
                    for a in range(3):
                        r = 2 * i + a - 1
                        if 0 <= r < S:
                            out[cl * S + r, col0 + cl * So + i] = w[c0 + cl, 0, a, b]
    return out


def _pack_bias(bias, cgs, So, rows):
    out = np.zeros((rows, len(cgs)), np.float32)
    for gi, (c0, nch) in enumerate(cgs):
        for cl in range(nch):
            out[cl * So:(cl + 1) * So, gi] = bias[c0 + cl]
    return out


def _host_consts(inp):
    c = {}
    c["bands1"] = _pack_bands(inp["w1"], 64, CG1, 64, 128)
    c["bands2"] = _pack_bands(inp["w2"], 32, CG2, 64, 128)
    c["bands3"] = _pack_bands(inp["w3"], 16, CG3, 64, 128)
    c["bands4"] = _pack_bands(inp["w4"], 8, [(0, 13)], 52, 104)
    c["bands5"] = _pack_bands(inp["w5"], 4, [(0, 13)], 26, 52)
    b6m = np.zeros((26, 26), np.float32)
    for bb in range(2):
        for ch in range(13):
            for r in range(2):
                b6m[ch * 2 + r, bb * 13 + ch] = inp["w6"][ch, 0, r, bb]
    c["bands6"] = b6m
    c["bv1"] = _pack_bias(inp["b1"], CG1, 32, 64)
    c["bv2"] = _pack_bias(inp["b2"], CG2, 16, 64)
    c["bv3"] = _pack_bias(inp["b3"], CG3, 8, 64)
    c["bv4"] = _pack_bias(inp["b4"], [(0, 13)], 4, 52)
    c["bv5"] = _pack_bias(inp["b5"], [(0, 13)], 2, 26)
    c["ubias"] = (2.5 * (inp["b6"] + 2.2)).astype(np.float32).reshape(13, 1)

    # KAN layer 1 rhs pieces: silu part [13,20]; basis+bias part [105,20]
    c["c1a"] = np.ascontiguousarray(inp["sb1"].astype(np.float32))
    c1b = np.zeros((105, HIDDEN), np.float32)
    for n in range(8):
        for i in range(13):
            c1b[n * 13 + i] = inp["coef1"][i, :, n] * inp["ss1"][i] / 6.0
    c1b[104] = inp["bias1"]
    c["c1b"] = c1b
    # KAN layer 2 rhs pieces: silu [20,10]; basis rows 0..107; rows 108..159 + bias
    c["c2s"] = np.ascontiguousarray(inp["sb2"].astype(np.float32))
    c2b = np.zeros((161, NCLS), np.float32)
    for n in range(8):
        for i in range(20):
            c2b[n * 20 + i] = inp["coef2"][i, :, n] * inp["ss2"][i] / 6.0
    c2b[160] = inp["bias2"]
    c["c2b1"] = np.ascontiguousarray(c2b[0:108])
    c["c2b2"] = np.ascontiguousarray(c2b[108:161])
    misc = np.zeros((128, 3), np.float32)
    misc[:, 0] = -2.2
    misc[:, 1] = 5.5
    c["misc"] = misc
    c["negk"] = np.tile(-np.arange(12, dtype=np.float32), (128, 1))
    c["iden"] = np.eye(128, dtype=np.float32)

    # pack into 4 DMA-able blocks
    def pack(layout, ncols, dt):
        out = np.zeros((128, ncols), dt)
        for k, (c0, nc_, rows) in layout.items():
            out[0:rows, c0:c0 + nc_] = c[k].astype(dt)
        return out

    return {"cea": pack(CEA, CEA_N, np.float32),
            "ceb": pack(CEB, CEB_N, BF16),
            "cec": pack(CEC, CEC_N, BF16),
            "ced": pack(CED, CED_N, np.float32)}


def _shard_x(x_shard):
    # [256,13,64,64] -> xa [16,6,128,16,64] (channel pairs), xb [16,64,16,64] (ch 12)
    xs = x_shard.reshape(NG, GI, 13, 64, 64)
    xa = xs[:, :, 0:12].transpose(0, 2, 3, 1, 4).reshape(NG, 6, 128, GI, 64)
    xb = xs[:, :, 12].transpose(0, 2, 1, 3)
    return np.ascontiguousarray(xa).astype(BF16), np.ascontiguousarray(xb).astype(BF16)


# ----------------------------------------------------------------------------
# bass program
# ----------------------------------------------------------------------------

def _build():
    global _BUILT
    if _BUILT is not None:
        return _BUILT
    from contextlib import ExitStack
    import concourse.bass as bass  # noqa: F401
    import concourse.bacc as bacc
    import concourse.tile as tile
    import concourse.mybir as mybir

    f32 = mybir.dt.float32
    bf16 = mybir.dt.bfloat16
    AF = mybir.ActivationFunctionType
    OP = mybir.AluOpType
    AX = mybir.AxisListType

    nc = bacc.Bacc("TRN2")
    T = nc.tensor

    d_xa = nc.dram_tensor("xa", [NG, 6, 128, GI, 64], bf16, kind="ExternalInput")
    d_xb = nc.dram_tensor("xb", [NG, 64, GI, 64], bf16, kind="ExternalInput")
    d_cea = nc.dram_tensor("cea", [128, CEA_N], f32, kind="ExternalInput")
    d_ceb = nc.dram_tensor("ceb", [128, CEB_N], bf16, kind="ExternalInput")
    d_cec = nc.dram_tensor("cec", [128, CEC_N], bf16, kind="ExternalInput")
    d_ced = nc.dram_tensor("ced", [128, CED_N], f32, kind="ExternalInput")
    d_out = nc.dram_tensor("out", [B, NCLS], f32, kind="ExternalOutput")

    with tile.TileContext(nc) as tc, ExitStack() as ctx:
        cpool = ctx.enter_context(tc.tile_pool(name="consts", bufs=1))
        t_cea = cpool.tile([128, CEA_N], f32, name="t_cea")
        t_ceb = cpool.tile([128, CEB_N], bf16, name="t_ceb")
        t_cec = cpool.tile([128, CEC_N], bf16, name="t_cec")
        t_ced = cpool.tile([128, CED_N], f32, name="t_ced")
        tcons = {}
        for tile_, layout in ((t_cea, CEA), (t_ceb, CEB), (t_cec, CEC),
                              (t_ced, CED)):
            for k, (c0, ncols, rows) in layout.items():
                tcons[k] = tile_[0:rows, c0:c0 + ncols]
        t_u = cpool.tile([13, B], f32, name="t_u")

        p_x1 = ctx.enter_context(tc.tile_pool(name="x1", bufs=3))
        p_x2 = ctx.enter_context(tc.tile_pool(name="x2", bufs=3))
        p_x3 = ctx.enter_context(tc.tile_pool(name="x3", bufs=3))
        p_sm = ctx.enter_context(tc.tile_pool(name="xsm", bufs=3))
        kpool = ctx.enter_context(tc.tile_pool(name="kan", bufs=3))
        X2, X3, X4, X5, X6 = {}, {}, {}, {}, {}
        KS = [{} for _ in range(NCH)]  # per-chain KAN state

        with tc.tile_pool(name="ps1", bufs=2, space="PSUM") as pp1, \
             tc.tile_pool(name="ps2", bufs=3, space="PSUM") as pp2, \
             tc.tile_pool(name="psk", bufs=3, space="PSUM") as ppk:
            # bias+relu PSUM->SBUF drain, spread across ACT/DVE/Pool so no
            # single engine's drain rate gates the PE matmul pipeline
            def brelu(eng, dest, src, bias_ap):
                if eng == 0:
                    nc.scalar.activation(dest, src, AF.Relu, bias=bias_ap)
                elif eng == 1:
                    nc.vector.tensor_scalar(dest, src, bias_ap, 0.0,
                                            op0=OP.add, op1=OP.max)
                else:
                    nc.gpsimd.tensor_scalar(dest, src, bias_ap, 0.0,
                                            op0=OP.add, op1=OP.max)

            def conv1(g, chunked=False):
                xt = p_x1.tile([128, 6 * GI * 64], bf16, tag="x1", name="xt")
                xbt = p_x1.tile([64, GI * 64], bf16, tag="x1b", name="xbt")
                if chunked:
                    for cc in range(6):
                        nc.sync.dma_start(
                            xt[:, cc * GI * 64:(cc + 1) * GI * 64]
                            .rearrange("p (i w) -> p i w", w=64),
                            d_xa[g, cc])
                else:
                    nc.sync.dma_start(
                        xt[:, :].rearrange("p (c i w) -> p c (i w)", c=6, i=GI),
                        d_xa[g].rearrange("c p i w -> p c (i w)"))
                nc.sync.dma_start(
                    xbt[:, :].rearrange("p (i w) -> p i w", w=64),
                    d_xb[g, :, :, :])
                x2t = [p_x2.tile([128, GI * 32], bf16, tag=f"x2_{k}",
                                 name=f"x2_{k}") for k in range(4)]
                for cg, (c0, nch) in enumerate(CG1):
                    K, M = nch * 64, nch * 32
                    if nch == 2:
                        xv = xt[0:K, cg * GI * 64:(cg + 1) * GI * 64]
                    else:
                        xv = xbt[0:K, :]
                    xv = xv.rearrange("p (i w) -> p i w", w=64)
                    ps = pp1.tile([64, GI * 32], f32, tag="ps1", name="ps1t")
                    pv = ps[0:M, :].rearrange("p (i w) -> p i w", w=32)
                    lo = lambda b: (cg * 3 + b) * 64
                    T.matmul(pv, tcons["bands1"][0:K, lo(1):lo(1) + M],
                             xv[:, :, 0:64:2], start=True, stop=False)
                    T.matmul(pv, tcons["bands1"][0:K, lo(2):lo(2) + M],
                             xv[:, :, 1:64:2], start=False, stop=False)
                    T.matmul(pv[:, :, 1:32], tcons["bands1"][0:K, lo(0):lo(0) + M],
                             xv[:, :, 1:62:2], start=False, stop=True,
                             skip_group_check=True)
                    dest = x2t[cg // 2][64 * (cg % 2):64 * (cg % 2) + M, :]
                    bap = tcons["bv1"][0:M, cg:cg + 1]
                    brelu((0, 1, 2, 0, 1, 2, 0)[cg], dest, ps[0:M, :], bap)
                X2[g] = x2t

            def conv2(g):
                x2t = X2.pop(g)
                x3t = [p_x3.tile([128, GI * 16], bf16, tag=f"x3_{k}",
                                 name=f"x3_{k}") for k in range(2)]
                for k4, (c0, nch) in enumerate(CG2):
                    K, M = nch * 32, nch * 16
                    xv = x2t[k4][0:K, :].rearrange("p (i w) -> p i w", w=32)
                    ps = pp2.tile([64, GI * 16], f32, tag="ps2", name="ps2t")
                    pv = ps[0:M, :].rearrange("p (i w) -> p i w", w=16)
                    lo = lambda b: (k4 * 3 + b) * 64
                    T.matmul(pv, tcons["bands2"][0:K, lo(1):lo(1) + M],
                             xv[:, :, 0:32:2], start=True, stop=False)
                    T.matmul(pv, tcons["bands2"][0:K, lo(2):lo(2) + M],
                             xv[:, :, 1:32:2], start=False, stop=False)
                    T.matmul(pv[:, :, 1:16], tcons["bands2"][0:K, lo(0):lo(0) + M],
                             xv[:, :, 1:30:2], start=False, stop=True,
                             skip_group_check=True)
                    dest = x3t[k4 // 2][64 * (k4 % 2):64 * (k4 % 2) + M, :]
                    bap = tcons["bv2"][0:M, k4:k4 + 1]
                    brelu((0, 1, 2, 0)[k4], dest, ps[0:M, :], bap)
                X3[g] = x3t

            def conv3(g):
                x3t = X3.pop(g)
                x4 = p_sm.tile([104, GI * 8], bf16, tag="x4", name="x4")
                for k8, (c0, nch) in enumerate(CG3):
                    K, M = nch * 16, nch * 8
                    xv = x3t[k8][0:K, :].rearrange("p (i w) -> p i w", w=16)
                    ps = pp2.tile([64, GI * 8], f32, tag="ps2", name="ps3t")
                    pv = ps[0:M, :].rearrange("p (i w) -> p i w", w=8)
                    lo = lambda b: (k8 * 3 + b) * 64
                    T.matmul(pv, tcons["bands3"][0:K, lo(1):lo(1) + M],
                             xv[:, :, 0:16:2], start=True, stop=False)
                    T.matmul(pv, tcons["bands3"][0:K, lo(2):lo(2) + M],
                             xv[:, :, 1:16:2], start=False, stop=False)
                    T.matmul(pv[:, :, 1:8], tcons["bands3"][0:K, lo(0):lo(0) + M],
                             xv[:, :, 1:14:2], start=False, stop=True,
                             skip_group_check=True)
                    dest = x4[64 * k8:64 * k8 + M, :]
                    bap = tcons["bv3"][0:M, k8:k8 + 1]
                    brelu((1, 2)[k8], dest, ps[0:M, :], bap)
                X4[g] = x4

            def conv4(g):
                x4 = X4.pop(g)
                x5 = p_sm.tile([52, GI * 4], bf16, tag="x5", name="x5")
                xv = x4[0:104, :].rearrange("p (i w) -> p i w", w=8)
                ps4 = pp2.tile([64, GI * 4], f32, tag="ps2", name="ps4t")
                pv = ps4[0:52, :].rearrange("p (i w) -> p i w", w=4)
                T.matmul(pv, tcons["bands4"][0:104, 52:104], xv[:, :, 0:8:2],
                         start=True, stop=False)
                T.matmul(pv, tcons["bands4"][0:104, 104:156], xv[:, :, 1:8:2],
                         start=False, stop=False)
                T.matmul(pv[:, :, 1:4], tcons["bands4"][0:104, 0:52],
                         xv[:, :, 1:6:2], start=False, stop=True,
                         skip_group_check=True)
                brelu(0, x5[:, :], ps4[0:52, :], tcons["bv4"][0:52, 0:1])
                X5[g] = x5

            def conv5(g):
                x5 = X5.pop(g)
                x6 = p_sm.tile([26, GI * 2], bf16, tag="x6", name="x6")
                xv = x5[0:52, :].rearrange("p (i w) -> p i w", w=4)
                ps5 = pp2.tile([64, GI * 2], f32, tag="ps2", name="ps5t")
                pv = ps5[0:26, :].rearrange("p (i w) -> p i w", w=2)
                T.matmul(pv, tcons["bands5"][0:52, 26:52], xv[:, :, 0:4:2],
                         start=True, stop=False)
                T.matmul(pv, tcons["bands5"][0:52, 52:78], xv[:, :, 1:4:2],
                         start=False, stop=False)
                T.matmul(pv[:, :, 1:2], tcons["bands5"][0:52, 0:26],
                         xv[:, :, 1:2:2], start=False, stop=True,
                         skip_group_check=True)
                brelu(1, x6[:, :], ps5[0:26, :], tcons["bv5"][0:26, 0:1])
                X6[g] = x6

            def conv6(g):
                x6 = X6.pop(g)
                xv = x6[0:26, :].rearrange("p (i w) -> p i w", w=2)
                ps6 = pp2.tile([64, GI], f32, tag="ps2", name="ps6t")
                T.matmul(ps6[0:13, :], tcons["bands6"][0:26, 0:13],
                         xv[:, :, 0:1], start=True, stop=False)
                T.matmul(ps6[0:13, :], tcons["bands6"][0:26, 13:26],
                         xv[:, :, 1:2], start=False, stop=True)
                nc.scalar.activation(t_u[:, g * GI:(g + 1) * GI], ps6[0:13, :],
                                     AF.Identity, bias=tcons["ubias"][0:13, 0:1],
                                     scale=2.5)

            # ---------------- KAN chain stages (chain c: images c*CL..) ------
            # PE-emitting pieces run right after conv1(w)'s matmuls; their
            # inputs come from elementwise pieces of earlier waves. Elementwise
            # pieces are emitted after the wave's conv drains.
            def kA_pe(c):
                s = KS[c]
                n = CH_N[c]
                sl = slice(CH_START[c], CH_START[c] + n)
                s["sl"], s["n"] = sl, n
                ps_uT = ppk.tile([128, 128], f32, tag="kps", name="ps_uT")
                T.transpose(ps_uT[0:n, 0:13], t_u[0:13, sl],
                            tcons["iden"][0:13, 0:13])
                s["uT"] = ps_uT

            def silu_stack(key, src_ap, nrows, n):
                # silu(0.4*u - 2.2) via exp + fast reciprocal: keeps ACT inside
                # the exp function-table set (Silu would force ~4us table swaps)
                a = kpool.tile([nrows, n], f32, tag=f"sa{key}", name="sa")
                nc.scalar.activation(a[:, :], src_ap, AF.Identity,
                                     bias=tcons["misc"][0:nrows, 0:1],
                                     scale=H_GRID)
                e = kpool.tile([nrows, n], f32, tag=f"se{key}", name="se")
                nc.scalar.activation(e[:, :], a[:, :], AF.Exp, scale=-1.0)
                p = kpool.tile([nrows, n], f32, tag=f"sp{key}", name="sp")
                nc.vector.tensor_scalar(p[:, :], e[:, :], 1.0, None, op0=OP.add)
                r = kpool.tile([nrows, n], f32, tag=f"sr{key}", name="sr")
                nc.vector.reciprocal_approx_fast(r[:, :], p[:, :])
                stk = kpool.tile([nrows, n], f32, tag=f"sk{key}", name="stk")
                nc.gpsimd.tensor_mul(stk[:, :], a[:, :], r[:, :])
                return stk

            def kA_el(c):
                s = KS[c]
                ps_uT = s["uT"]
                s["stkA"] = silu_stack("1", t_u[0:13, s["sl"]], 13)
                # truncated-power cubic basis: 6*B_n(u) = sum_k (-1)^k C(4,k)
                # relu(u-n-k)^3 -- replaces the Cox-de-Boor level recursion
                R = kpool.tile([CL, 156], f32, tag="R1", name="Rt")
                for k in range(12):
                    dst = R[:, k * 13:(k + 1) * 13]
                    if k % 3 == 0:
                        nc.scalar.activation(dst, ps_uT[0:CL, 0:13], AF.Relu,
                                             bias=tcons["negk"][0:CL, k:k + 1])
                    elif k % 3 == 1:
                        nc.vector.tensor_scalar(dst, ps_uT[0:CL, 0:13],
                                                float(-k), 0.0,
                                                op0=OP.add, op1=OP.max)
                    else:
                        nc.gpsimd.tensor_scalar(dst, ps_uT[0:CL, 0:13],
                                                float(-k), 0.0,
                                                op0=OP.add, op1=OP.max)
                sq = kpool.tile([CL, 156], f32, tag="sq1", name="sq")
                nc.vector.tensor_mul(sq[:, :], R[:, :], R[:, :])
                cu = kpool.tile([CL, 156], f32, tag="cu1", name="cu")
                nc.gpsimd.tensor_mul(cu[:, :], sq[:, :], R[:, :])
                p1 = kpool.tile([CL, 104], f32, tag="p11", name="p1")
                nc.vector.scalar_tensor_tensor(p1[:, :], cu[:, 26:130], 6.0,
                                               cu[:, 0:104],
                                               op0=OP.mult, op1=OP.add)
                ns = kpool.tile([CL, 104], f32, tag="ns1", name="ns")
                nc.gpsimd.tensor_add(ns[:, :], cu[:, 13:117], cu[:, 39:143])
                po = kpool.tile([CL, 104], f32, tag="po1", name="po")
                nc.vector.tensor_add(po[:, :], p1[:, :], cu[:, 52:156])
                Bc = kpool.tile([CL, 105], f32, tag="B3", name="Bc")
                nc.gpsimd.scalar_tensor_tensor(Bc[:, 0:104], ns[:, :], -4.0,
                                               po[:, :], op0=OP.mult, op1=OP.add)
                nc.vector.memset(Bc[:, 104:105], 1.0)
                s["Bc3"] = Bc

            def kC_pe(c):
                s = KS[c]
                ps_b1 = ppk.tile([128, 128], f32, tag="kps", name="ps_b1")
                T.transpose(ps_b1[0:105, 0:CL], s["Bc3"][:, 0:105],
                            tcons["iden"][0:CL, 0:CL])
                s["b1"] = ps_b1

            def kC_el(c):
                s = KS[c]
                stkB = kpool.tile([105, CL], f32, tag="stkB", name="stkB")
                nc.scalar.activation(stkB[:, :], s["b1"][0:105, 0:CL], AF.Copy)
                s["stkB"] = stkB

            def kD_pe(c):  # layer-1 matmul
                s = KS[c]
                ps_h1 = ppk.tile([128, 128], f32, tag="kps", name="ps_h1")
                T.matmul(ps_h1[0:CL, 0:20], s["stkA"][:, :], tcons["c1a"][:, :],
                         start=True, stop=False)
                T.matmul(ps_h1[0:CL, 0:20], s["stkB"][:, :], tcons["c1b"][:, :],
                         start=False, stop=True)
                s["h1"] = ps_h1

            def kD_el(c):
                s = KS[c]
                u2 = kpool.tile([CL, 20], f32, tag="u2", name="u2")
                nc.scalar.activation(u2[:, :], s["h1"][0:CL, 0:20], AF.Identity,
                                     bias=tcons["misc"][0:CL, 1:2], scale=2.5)
                s["u2"] = u2

            def kE_pe(c):
                s = KS[c]
                ps_t2 = ppk.tile([128, 128], f32, tag="kps", name="ps_t2")
                T.transpose(ps_t2[0:20, 0:CL], s["u2"][:, :],
                            tcons["iden"][0:CL, 0:CL])
                s["t2"] = ps_t2

            def kE_el(c):
                s = KS[c]
                u2 = s["u2"]
                s["stk2s"] = silu_stack("2", s["t2"][0:20, 0:CL], 20)
                R2 = kpool.tile([CL, 240], f32, tag="R2", name="R2t")
                for k in range(12):
                    dst = R2[:, k * 20:(k + 1) * 20]
                    if k % 3 == 0:
                        nc.scalar.activation(dst, u2[:, :], AF.Relu,
                                             bias=tcons["negk"][0:CL, k:k + 1])
                    elif k % 3 == 1:
                        nc.vector.tensor_scalar(dst, u2[:, :], float(-k), 0.0,
                                                op0=OP.add, op1=OP.max)
                    else:
                        nc.gpsimd.tensor_scalar(dst, u2[:, :], float(-k), 0.0,
                                                op0=OP.add, op1=OP.max)
                sq2 = kpool.tile([CL, 240], f32, tag="sq2", name="sq2")
                nc.vector.tensor_mul(sq2[:, :], R2[:, :], R2[:, :])
                cu2 = kpool.tile([CL, 240], f32, tag="cu2", name="cu2")
                nc.gpsimd.tensor_mul(cu2[:, :], sq2[:, :], R2[:, :])
                p12 = kpool.tile([CL, 160], f32, tag="p12", name="p12")
                nc.vector.scalar_tensor_tensor(p12[:, :], cu2[:, 40:200], 6.0,
                                               cu2[:, 0:160],
                                               op0=OP.mult, op1=OP.add)
                ns2 = kpool.tile([CL, 160], f32, tag="ns2", name="ns2")
                nc.gpsimd.tensor_add(ns2[:, :], cu2[:, 20:180], cu2[:, 60:220])
                po2 = kpool.tile([CL, 160], f32, tag="po2", name="po2")
                nc.vector.tensor_add(po2[:, :], p12[:, :], cu2[:, 80:240])
                Bc2 = kpool.tile([CL, 161], f32, tag="B3_2", name="Bc2")
                nc.gpsimd.scalar_tensor_tensor(Bc2[:, 0:160], ns2[:, :], -4.0,
                                               po2[:, :], op0=OP.mult, op1=OP.add)
                nc.vector.memset(Bc2[:, 160:161], 1.0)
                s["Bc2f"] = Bc2

            def kG_pe(c):
                s = KS[c]
                Bc2 = s["Bc2f"]
                ps_b2 = ppk.tile([128, 128], f32, tag="kps", name="ps_b2")
                T.transpose(ps_b2[0:108, 0:CL], Bc2[:, 0:108],
                            tcons["iden"][0:CL, 0:CL])
                ps_b3 = ppk.tile([128, 128], f32, tag="kps", name="ps_b3")
                T.transpose(ps_b3[0:53, 0:CL], Bc2[:, 108:161],
                            tcons["iden"][0:CL, 0:CL])
                s["b2"], s["b3"] = ps_b2, ps_b3

            def kG_el(c):
                s = KS[c]
                stk2a = kpool.tile([108, CL], f32, tag="s2a", name="stk2a")
                stk2b = kpool.tile([53, CL], f32, tag="s2b", name="stk2b")
                nc.scalar.activation(stk2a[:, :], s["b2"][0:108, 0:CL], AF.Copy)
                nc.scalar.activation(stk2b[:, :], s["b3"][0:53, 0:CL], AF.Copy)
                s["stk2a"], s["stk2b"] = stk2a, stk2b

            def kH_pe(c):  # layer-2 matmul -> logits
                s = KS[c]
                ps_lg = ppk.tile([128, 128], f32, tag="kps", name="ps_lg")
                T.matmul(ps_lg[0:CL, 0:NCLS], s["stk2a"][:, :], tcons["c2b1"][:, :],
                         start=True, stop=False)
                T.matmul(ps_lg[0:CL, 0:NCLS], s["stk2s"][:, :], tcons["c2s"][:, :],
                         start=False, stop=False)
                T.matmul(ps_lg[0:CL, 0:NCLS], s["stk2b"][:, :], tcons["c2b2"][:, :],
                         start=False, stop=True)
                s["lg"] = ps_lg

            def kH_el(c):
                s = KS[c]
                lg_s = kpool.tile([CL, NCLS], f32, tag="lg_s", name="lg_s")
                nc.vector.tensor_copy(lg_s[:, :], s["lg"][0:CL, 0:NCLS])
                s["lg_s"] = lg_s
                negm = kpool.tile([CL, 1], f32, tag="negm", name="negm")
                nc.vector.reduce_max(negm[:, :], lg_s[:, :], axis=AX.X,
                                     negate=True)
                s["negm"] = negm
                ex = kpool.tile([CL, NCLS], f32, tag="ex", name="ex")
                nc.scalar.activation(ex[:, :], lg_s[:, :], AF.Exp,
                                     bias=negm[:, 0:1])
                ssum = kpool.tile([CL, 1], f32, tag="ssum", name="ssum")
                nc.vector.reduce_sum(ssum[:, :], ex[:, :], axis=AX.X)
                s["ssum"] = ssum

            def kan_ln_tail(c):
                # only Ln lives outside the exp function-table set; deferring
                # these 4 ops to the end means exactly one table load
                s = KS[c]
                lsum = kpool.tile([CL, 1], f32, tag="lsum", name="lsum")
                nc.scalar.activation(lsum[:, :], s["ssum"][:, :], AF.Ln,
                                     bias=tcons["misc"][0:CL, 2:3])
                s["lsum"] = lsum

            def kan_out(c):
                s = KS[c]
                res = kpool.tile([CL, NCLS], f32, tag="res", name="res")
                nc.gpsimd.tensor_scalar(res[:, :], s["lg_s"][:, :],
                                        s["negm"][:, 0:1], s["lsum"][:, 0:1],
                                        op0=OP.add, op1=OP.subtract)
                nc.sync.dma_start(d_out[s["sl"], :], res[:, :])

            # chain c's stage schedule: {wave: [(pe_fn, el_fn, c), ...]}
            kan_sched = {}
            for c in range(NCH):
                w0 = 4 * c + 9
                for dw, pe_fn, el_fn in ((0, kA_pe, kA_el), (1, kC_pe, kC_el),
                                         (2, kD_pe, kD_el), (3, kE_pe, kE_el),
                                         (4, kG_pe, kG_el), (5, kH_pe, kH_el)):
                    kan_sched.setdefault(w0 + dw, []).append((pe_fn, el_fn, c))

            # ---- emission: packed consts + software-pipelined conv waves ----
            nc.sync.dma_start(t_cea[:, :], d_cea[:, :])
            nc.sync.dma_start(t_ceb[:, :], d_ceb[:, :])
            conv1(0, chunked=True)
            nc.sync.dma_start(t_cec[:, :], d_cec[:, :])
            nc.sync.dma_start(t_ced[:, :], d_ced[:, :])

            n_waves = max(NG + 5, max(kan_sched) + 1)
            for w in range(1, n_waves):
                if w < NG:
                    conv1(w, chunked=(w == 1))
                for pe_fn, el_fn, c in kan_sched.get(w, []):
                    if pe_fn is not None:
                        pe_fn(c)
                for pe_fn, el_fn, c in kan_sched.get(w, []):
                    if el_fn is not None:
                        el_fn(c)
                if 1 <= w < NG + 1:
                    conv2(w - 1)
                if 2 <= w < NG + 2:
                    conv3(w - 2)
                if 3 <= w < NG + 3:
                    conv4(w - 3)
                if 4 <= w < NG + 4:
                    conv5(w - 4)
                if 5 <= w < NG + 5:
                    conv6(w - 5)

            # deferred Ln + output (one Exp->Ln table load for the program)
            for c in range(NCH):
                kan_ln_tail(c)
            for c in range(NCH):
                kan_out(c)

    nc.compile()  # bacc lowering: wait splitting via event semaphores, etc.
    _BUILT = (nc, None)
    return _BUILT


# ----------------------------------------------------------------------------
# entry point
# ----------------------------------------------------------------------------

LAST_RESULTS = None  # BassKernelResults of the most recent run (for test.py)


def kernel(**inputs):
    global LAST_RESULTS
    from concourse import bass_utils

    x = np.asarray(inputs["x"], np.float32)
    cons = _host_consts({k: np.asarray(v, np.float32)
                         for k, v in inputs.items() if k != "x"})
    nc, _names = _build()

    in_maps = []
    for core in range(NCORE):
        xa, xb = _shard_x(x[core * B:(core + 1) * B])
        in_maps.append({"xa": xa, "xb": xb, **cons})
    res = bass_utils.run_bass_kernel_spmd(nc, in_maps, core_ids=list(range(NCORE)))
    LAST_RESULTS = res
    return np.concatenate([r["out"] for r in res.results], axis=0)
